# revision 56
# baseline (speedup 1.0000x reference)
"""MoRALayer Trainium2 kernel.

Data-parallel over 8 NeuronCores (2048 samples each). Per core, feature-major
layout ([feature, sample] tiles): 10-step LSTM ACT loop, 2 sparse-MoE levels
(top-2 of 4 experts), LayerNorm.

Precision scheme (validated against a numpy rounding simulator of the MoE
top-2 routing flip rate): xWi, gate logits and level-0 experts use a hi/lo
tf32 (float32r) split - 3 single-cycle-per-row matmuls recovering ~fp32
accuracy - because their noise feeds the routing decisions (level-0 expert
output drives level-1 gates, whose top2/3rd margins are ~10x tighter than
level-0's). The dominant LSTM recurrence (h@Wh, 10 steps) and the level-1
experts run single-pass tf32: their ~5e-4 noise costs only ~1 routing flip
out of 16k samples (rel err ~7e-3 vs the 2e-2 gate) and cuts tensor time 3x.
Elementwise math and transcendentals are fp32.

Structure: xwi precomputed once (block-major DRAM layout, 4KB lines), held
SBUF-resident per 256-sample block across all 10 LSTM steps; two blocks run
the LSTM in tandem (interleaved step emission, per-parity state) so one
block's cross-engine activation chains hide under the other's matmuls; i/f
and g/o gate pre-activations share packed PSUM banks; SBUF-only elementwise
ops offload to the idle GpSimd engine except on the step-critical last chunk;
per-step halt projections are deferred one step off the inter-step critical
path. Both MoE levels fuse their epilogue into the last expert's evacuation
via partial-sum readback: level 0 rounds/splits the l1 input inline, level 1
runs LayerNorm per block in sample-major layout (transpose first, then
per-partition-scalar normalization), so the LN tail overlaps expert compute.
"""
import os
import sys
import numpy as np

for p in ("/root/.axon_site", "/root/.axon_site/_ro/trn_rl_repo",
          "/root/.axon_site/_ro/pypackages", "/opt/trn_rl_repo"):
    if os.path.isdir(p) and p not in sys.path:
        sys.path.append(p)

import concourse.bass as bass
import concourse.mybir as mybir
import concourse.tile as tile
from concourse import bacc
from concourse.bass_utils import run_bass_kernel_spmd

dt = mybir.dt
AF = mybir.ActivationFunctionType
ALU = mybir.AluOpType
AX = mybir.AxisListType

B, D, H, E, LV, TOPK, T = 16384, 768, 768, 4, 2, 2, 10
LN_EPS = 1e-5
NCORES = 8
BPC = B // NCORES          # samples per core
NBS = 256                  # samples per block
NB = BPC // NBS            # blocks per core
KC = D // 128              # 6 k-chunks over features
GM = 4 * H // 128          # 24 gate m-chunks

F32, F32R = dt.float32, dt.float32r


def _tf32(x):
    u = np.ascontiguousarray(x, np.float32).view(np.uint32)
    u = (u + np.uint32(0x1000)) & np.uint32(0xFFFFE000)
    return u.view(np.float32)


def _split(x):
    hi = _tf32(x)
    lo = _tf32(np.asarray(x, np.float32) - hi)
    return hi, lo


class Builder:
    def __init__(self, nc, tc, ctxpools):
        self.nc = nc
        self.tc = tc

    # ---- helpers -------------------------------------------------------
    def mm_group(self, psum_ap, pairs, first=True, last=True):
        nc = self.nc
        n = len(pairs)
        for i, (l, r) in enumerate(pairs):
            nc.tensor.matmul(psum_ap, l, r,
                             start=(first and i == 0), stop=(last and i == n - 1))

    def decomp(self, pool, val_ap, tag, mode, eng=None):
        """Split an fp32 AP into matmul operand tiles per mode.
        Returns list of rhs aps: hilo -> [hi, lo]; tf32 -> [hi]; fp32 -> [val].
        The hi+lo pair represents val exactly regardless of the engine's
        rounding mode, so eng may be gpsimd to offload the DVE."""
        nc = self.nc
        v = eng if eng is not None else nc.vector
        if mode == "fp32":
            return [val_ap]
        hi = pool.tile(list(val_ap.shape), F32R, tag=tag + "_hi", bufs=1)
        v.tensor_copy(hi[:], val_ap)
        if mode == "tf32":
            return [hi[:]]
        lo = pool.tile(list(val_ap.shape), F32R, tag=tag + "_lo", bufs=1)
        v.tensor_sub(lo[:], val_ap, hi[:].bitcast(F32))
        return [hi[:], lo[:]]

    @staticmethod
    def pairs(w_terms, r_terms):
        """w_terms/r_terms: lists from weight dict / decomp. hilo x hilo ->
        (wh,rh),(wh,rl),(wl,rh); single x single -> 1 pair; fp32 -> 1."""
        if len(w_terms) == 2 and len(r_terms) == 2:
            return [(w_terms[0], r_terms[0]), (w_terms[0], r_terms[1]),
                    (w_terms[1], r_terms[0])]
        if len(w_terms) == 1 and len(r_terms) == 1:
            return [(w_terms[0], r_terms[0])]
        if len(w_terms) == 2 and len(r_terms) == 1:
            return [(w_terms[0], r_terms[0]), (w_terms[1], r_terms[0])]
        if len(w_terms) == 1 and len(r_terms) == 2:
            return [(w_terms[0], r_terms[0]), (w_terms[0], r_terms[1])]
        raise ValueError


def build(cfg):
    mode = cfg.get("mode", "hilo")          # xWi/gates/l0 experts
    mode_l1 = cfg.get("mode_l1", "tf32")    # l1 expert matmuls
    mode_lstm = cfg.get("mode_lstm", mode)  # recurrent h@Wh + halt matmuls
    nb = cfg.get("nb", NB)
    bpc = nb * NBS

    nc = bacc.Bacc("TRN2", target_bir_lowering=False, debug=False)
    names = []

    def inp(name, shape, dtyp=F32):
        names.append(name)
        return nc.dram_tensor(name, list(shape), dtyp, kind="ExternalInput").ap()

    two = 2 if mode == "hilo" else 1
    two1 = 2 if mode_l1 == "hilo" else 1
    two_l = 2 if mode_lstm == "hilo" else 1
    wdt_l = F32R if mode_lstm != "fp32" else F32
    # inputs (weights host-transposed; hi/lo host-split when hilo)
    x_d = inp("x", [bpc, D])
    wi_d = inp("wiT", [two, D, 4 * H], F32R if mode != "fp32" else F32)
    wh_d = inp("whT", [two_l, D, 4 * H], wdt_l)
    whalt_d = inp("whaltT", [two_l, D, 1], wdt_l)
    bib_d = inp("bib", [4 * H, 1])         # bi + bh
    bhalt_d = inp("bhalt", [1, 1])
    gate_d = inp("gateT", [LV, two, D, E], F32R if mode != "fp32" else F32)
    gateb_d = inp("gateb", [LV, 1, E])
    w1l0_d = inp("w1l0T", [E, two, H, H], F32R if mode != "fp32" else F32)
    w2l0_d = inp("w2l0T", [E, two, H, D], F32R if mode != "fp32" else F32)
    w1l1_d = inp("w1l1T", [E, two1, H, H], F32R if mode_l1 != "fp32" else F32)
    w2l1_d = inp("w2l1T", [E, two1, H, D], F32R if mode_l1 != "fp32" else F32)
    b1_d = inp("b1", [LV, E, H, 1])
    b2s_d = inp("b2s", [LV, E, D])         # stacked for k=4 matmul
    gammaT_d = inp("gammaT", [1, D])
    betaT_d = inp("betaT", [1, D])
    ident_d = inp("ident128", [128, 128])
    out_d = nc.dram_tensor("out", [bpc, D], F32, kind="ExternalOutput").ap()

    # DRAM scratch (xwi block-major: per-partition lines of 4*NBS floats)
    xwi_d = nc.dram_tensor("xwi_s", [KC, 128, nb, 4, NBS], F32).ap()
    avg_d = [nc.dram_tensor(f"avg_s{i}", [D, bpc], F32R if mode != "fp32" else F32).ap()
             for i in range(two)]
    l0o_d = nc.dram_tensor("l0o_s", [D, bpc], F32).ap()
    l1o_d = nc.dram_tensor("l1o_s", [D, bpc], F32).ap()

    wdt = F32R if mode != "fp32" else F32
    wdt1 = F32R if mode_l1 != "fp32" else F32

    with tile.TileContext(nc) as tc:
        bld = Builder(nc, tc, None)
        with tc.tile_pool(name="const", bufs=1) as cpool:
            ident = cpool.tile([128, 128], F32, tag="ident")
            nc.sync.dma_start(out=ident[:], in_=ident_d[:])
            ones1 = cpool.tile([1, 128], F32, tag="ones1")
            nc.vector.memset(ones1[:], 1.0)
            cst01 = cpool.tile([1, 128], F32, tag="cst01")
            nc.vector.memset(cst01[:], 1.0 / T)
            oinv = cpool.tile([128, 1], F32, tag="oinv")
            nc.vector.memset(oinv[:], 1.0 / D)
            eps_t = cpool.tile([1, 1], F32, tag="epsln")
            nc.vector.memset(eps_t[:], LN_EPS)
            bhalt_t = cpool.tile([1, 1], F32, tag="bhalt")
            nc.sync.dma_start(out=bhalt_t[:], in_=bhalt_d[:, :])
            whalt_t = []
            for s in range(two_l):
                ks = []
                for k in range(KC):
                    t = cpool.tile([128, 1], wdt_l, tag=f"whalt{s}_{k}")
                    nc.sync.dma_start(out=t[:], in_=whalt_d[s, k * 128:(k + 1) * 128, :])
                    ks.append(t)
                whalt_t.append(ks)
            # LN constants (broadcast along partitions for sample-major LN);
            # the staging tiles + psum close before phase A1 opens
            gammab = cpool.tile([128, D], F32, tag="gammab")
            betab = cpool.tile([128, D], F32, tag="betab")
            eps128 = cpool.tile([128, 1], F32, tag="eps128")
            nc.vector.memset(eps128[:], LN_EPS)
            with tc.tile_pool(name="lnst", bufs=1) as stp, \
                 tc.tile_pool(name="lnbc", bufs=2, space="PSUM") as bcp:
                gT = stp.tile([1, D], F32, tag="gT")
                nc.sync.dma_start(out=gT[:], in_=gammaT_d[:, :])
                bT = stp.tile([1, D], F32, tag="bT")
                nc.sync.dma_start(out=bT[:], in_=betaT_d[:, :])
                for j in range(0, D, 512):
                    w = min(512, D - j)
                    pb = bcp.tile([128, 512], F32, tag="bc")
                    nc.tensor.matmul(pb[:, :w], ones1[:], gT[:, j:j + w], start=True, stop=True)
                    nc.vector.tensor_copy(gammab[:, j:j + w], pb[:, :w])
                    pb2 = bcp.tile([128, 512], F32, tag="bc")
                    nc.tensor.matmul(pb2[:, :w], ones1[:], bT[:, j:j + w], start=True, stop=True)
                    nc.vector.tensor_copy(betab[:, j:j + w], pb2[:, :w])

            # ================= phase A1: xWi =================
            with tc.tile_pool(name="wiA", bufs=1) as wpool, \
                 tc.tile_pool(name="a1work", bufs=2) as work, \
                 tc.tile_pool(name="a1ps", bufs=3, space="PSUM") as pspool:
                wi_t = []
                for s in range(two):
                    ks = []
                    for k in range(KC):
                        t = wpool.tile([128, 4 * H], wdt, tag=f"wi{s}_{k}")
                        nc.sync.dma_start(out=t[:], in_=wi_d[s, k * 128:(k + 1) * 128, :])
                        ks.append(t)
                    wi_t.append(ks)
                bib_t = []
                for m in range(GM):
                    t = wpool.tile([128, 1], F32, tag=f"bib{m}", name=f"bib{m}")
                    nc.sync.dma_start(out=t[:], in_=bib_d[m * 128:(m + 1) * 128, :])
                    bib_t.append(t)
                for blk in range(nb):
                    s0 = blk * NBS
                    xsm = []
                    for sm in range(NBS // 128):
                        t = work.tile([128, D], F32, tag=f"xsm{sm}")
                        nc.sync.dma_start(out=t[:], in_=x_d[s0 + sm * 128:s0 + (sm + 1) * 128, :])
                        xsm.append(t)
                    xts = []
                    for k in range(KC):
                        xt = work.tile([128, NBS], F32, tag=f"xT{k}")
                        for sm in range(NBS // 128):
                            ptx = pspool.tile([128, 128], F32, tag="ptx")
                            nc.tensor.transpose(ptx[:], xsm[sm][:, k * 128:(k + 1) * 128], ident[:])
                            nc.vector.tensor_copy(xt[:, sm * 128:(sm + 1) * 128], ptx[:])
                        xts.append(bld.decomp(work, xt[:], f"xT{k}", mode))
                    for m in range(GM):
                        ps = pspool.tile([128, NBS], F32, tag="a1")
                        pr = []
                        for k in range(KC):
                            w_terms = [wi_t[s][k][:, m * 128:(m + 1) * 128] for s in range(two)]
                            pr += bld.pairs(w_terms, xts[k])
                        bld.mm_group(ps[:], pr)
                        ev = work.tile([128, NBS], F32, tag="a1ev")
                        nc.vector.tensor_scalar(ev[:], ps[:], bib_t[m][:], None, ALU.add)
                        nc.sync.dma_start(out=xwi_d[m % KC, :, blk, m // KC, :], in_=ev[:])

            # ================= phase A2: LSTM =================
            with tc.tile_pool(name="whA", bufs=1) as wpool, \
                 tc.tile_pool(name="xwip", bufs=2) as xpool, \
                 tc.tile_pool(name="state", bufs=1) as spool, \
                 tc.tile_pool(name="a2work", bufs=2) as work, \
                 tc.tile_pool(name="a2act", bufs=1) as apool, \
                 tc.tile_pool(name="a2ps", bufs=2, space="PSUM") as pspool, \
                 tc.tile_pool(name="a2psr", bufs=1, space="PSUM") as psrem, \
                 tc.tile_pool(name="a2psh", bufs=2, space="PSUM") as pshalt:
                wh_t = []
                for s in range(two_l):
                    ks = []
                    for k in range(KC):
                        t = wpool.tile([128, 4 * H], wdt_l, tag=f"wh{s}_{k}")
                        nc.sync.dma_start(out=t[:], in_=wh_d[s, k * 128:(k + 1) * 128, :])
                        ks.append(t)
                    wh_t.append(ks)

                hdt = F32 if mode_lstm == "fp32" else F32R

                def mk_ctx(par, blk):
                    # per-parity state: two blocks fly in tandem so one
                    # block's cross-engine chains hide under the other's mms
                    xwi_sb = xpool.tile([128, KC, 4, NBS], F32,
                                        tag=f"xwisb{par}", bufs=1)
                    nc.sync.dma_start(
                        out=xwi_sb[:],
                        in_=xwi_d[:, :, blk, :, :].rearrange("k p g s -> p k g s"))
                    tl = lambda shape, dty, tg: spool.tile(
                        shape, dty, tag=tg, name=tg)
                    return {
                        "s0": blk * NBS, "xwi": xwi_sb,
                        "c": [tl([128, NBS], F32, f"c{k}_{par}") for k in range(KC)],
                        "hs": [tl([128, NBS], F32, f"hs{k}_{par}") for k in range(KC)],
                        "hh": [[tl([128, NBS], hdt, f"hh{b}_{k}_{par}")
                                for k in range(KC)] for b in range(2)],
                        "hl": [[tl([128, NBS], F32R, f"hl{b}_{k}_{par}")
                                for k in range(KC)] for b in range(2)]
                              if mode_lstm == "hilo" else [[None] * KC] * 2,
                        "hp": tl([1, NBS], F32, f"hp{par}"),
                        "rem": tl([1, NBS], F32, f"rem{par}"),
                    }

                def h_terms(cx, k, b):
                    if mode_lstm == "hilo":
                        return [cx["hh"][b][k][:], cx["hl"][b][k][:]]
                    return [cx["hh"][b][k][:]]

                def store_h(cx, k, b, t_ap):
                    # the last chunk gates the next step's matmuls - keep its
                    # chain on the faster DVE
                    eng = nc.vector if k == KC - 1 else nc.gpsimd
                    eng.tensor_copy(cx["hh"][b][k][:], t_ap)
                    if mode_lstm == "hilo":
                        nc.vector.tensor_sub(cx["hl"][b][k][:], t_ap,
                                             cx["hh"][b][k][:].bitcast(F32))

                def halt_and_update(cx, step):
                    psh = pshalt.tile([1, NBS], F32, tag="halt")
                    pr = []
                    for k in range(KC):
                        w_terms = [whalt_t[s][k][:] for s in range(two_l)]
                        pr += bld.pairs(w_terms, h_terms(cx, k, step % 2))
                    bld.mm_group(psh[:], pr)
                    y = work.tile([1, NBS], F32, tag="y", bufs=1)
                    nc.scalar.activation(y[:], psh[:], AF.Sigmoid, bias=bhalt_t[:])
                    hp, rem = cx["hp"], cx["rem"]
                    if step == 0:
                        nc.vector.tensor_copy(hp[:], y[:])
                        nc.vector.tensor_scalar(rem[:], y[:], -1.0, 1.0, ALU.mult, ALU.add)
                    else:
                        t2 = work.tile([1, NBS], F32, tag="t2", bufs=1)
                        nc.vector.tensor_scalar(t2[:], hp[:], -1.0, 1.0, ALU.mult, ALU.add)
                        t3 = work.tile([1, NBS], F32, tag="t3", bufs=1)
                        nc.vector.tensor_mul(t3[:], t2[:], y[:])
                        nc.vector.tensor_add(hp[:], hp[:], t3[:])
                        t4 = work.tile([1, NBS], F32, tag="t4", bufs=1)
                        nc.vector.tensor_scalar(t4[:], hp[:], -1.0, 1.0, ALU.mult, ALU.add)
                        nc.vector.tensor_add(rem[:], rem[:], t4[:])

                def emit_step0(cx):
                    # h=c=0; gates = xwi directly
                    for k in range(KC):
                        sif = apool.tile([128, 2, NBS], F32, tag="sif", bufs=2)
                        nc.scalar.activation(sif[:, 0, :], cx["xwi"][:, k, 0, :], AF.Sigmoid)
                        tg = apool.tile([128, NBS], F32, tag="tg", bufs=2)
                        nc.scalar.activation(tg[:], cx["xwi"][:, k, 2, :], AF.Tanh)
                        so = apool.tile([128, NBS], F32, tag="so", bufs=2)
                        nc.scalar.activation(so[:], cx["xwi"][:, k, 3, :], AF.Sigmoid)
                        nc.vector.tensor_mul(cx["c"][k][:], sif[:, 0, :], tg[:])
                        th = work.tile([128, NBS], F32, tag="th")
                        nc.scalar.activation(th[:], cx["c"][k][:], AF.Tanh)
                        nc.vector.tensor_mul(th[:], so[:], th[:])
                        store_h(cx, k, 0, th[:])
                        nc.vector.tensor_copy(cx["hs"][k][:], th[:])

                def emit_step(cx, step):
                    for k in range(KC):
                        # gates i,f packed in one PSUM bank; g,o in another
                        ps_if = pspool.tile([128, 2, NBS], F32, tag="psif")
                        ps_go = pspool.tile([128, 2, NBS], F32, tag="psgo")
                        for j, gi in enumerate((0, 1)):
                            m = 6 * gi + k
                            pr = []
                            for kk in range(KC):
                                w_terms = [wh_t[s][kk][:, m * 128:(m + 1) * 128] for s in range(two_l)]
                                pr += bld.pairs(w_terms, h_terms(cx, kk, (step - 1) % 2))
                            bld.mm_group(ps_if[:, j, :], pr)
                        for j, gi in enumerate((2, 3)):
                            m = 6 * gi + k
                            pr = []
                            for kk in range(KC):
                                w_terms = [wh_t[s][kk][:, m * 128:(m + 1) * 128] for s in range(two_l)]
                                pr += bld.pairs(w_terms, h_terms(cx, kk, (step - 1) % 2))
                            bld.mm_group(ps_go[:, j, :], pr)
                        gif = work.tile([128, 2, NBS], F32, tag="gif")
                        nc.vector.tensor_add(gif[:], ps_if[:], cx["xwi"][:, k, 0:2, :])
                        sif = apool.tile([128, 2, NBS], F32, tag="sif", bufs=2)
                        nc.scalar.activation(sif[:], gif[:], AF.Sigmoid)
                        ggo = work.tile([128, 2, NBS], F32, tag="ggo")
                        nc.vector.tensor_add(ggo[:], ps_go[:], cx["xwi"][:, k, 2:4, :])
                        tg = apool.tile([128, NBS], F32, tag="tg", bufs=2)
                        nc.scalar.activation(tg[:], ggo[:, 0, :], AF.Tanh)
                        so = apool.tile([128, NBS], F32, tag="so", bufs=2)
                        nc.scalar.activation(so[:], ggo[:, 1, :], AF.Sigmoid)
                        t1 = work.tile([128, NBS], F32, tag="t1g", bufs=2)
                        teng = nc.vector if k == KC - 1 else nc.gpsimd
                        teng.tensor_mul(t1[:], sif[:, 0, :], tg[:])
                        # k<3: whole c-chain stays in-order on GpSimd (it has
                        # no PSUM port, so the gate sums themselves cannot move)
                        ceng = nc.gpsimd if k < 3 else nc.vector
                        ceng.tensor_mul(cx["c"][k][:], cx["c"][k][:], sif[:, 1, :])
                        ceng.tensor_add(cx["c"][k][:], cx["c"][k][:], t1[:])
                        th = work.tile([128, NBS], F32, tag="th")
                        nc.scalar.activation(th[:], cx["c"][k][:], AF.Tanh)
                        nc.vector.tensor_mul(th[:], so[:], th[:])
                        store_h(cx, k, step % 2, th[:])
                        nc.gpsimd.tensor_add(cx["hs"][k][:], cx["hs"][k][:], th[:])

                def emit_avg(cx):
                    # avg = rem * hsum / T -> DRAM (hi/lo)
                    psb = psrem.tile([128, NBS], F32, tag="remb")
                    nc.tensor.matmul(psb[:], cst01[:], cx["rem"][:], start=True, stop=True)
                    for k in range(KC):
                        av = work.tile([128, NBS], F32, tag="av", bufs=2)
                        nc.vector.tensor_mul(av[:], cx["hs"][k][:], psb[:])
                        terms = bld.decomp(work, av[:], "avd", mode, eng=nc.gpsimd)
                        for s, tm in enumerate(terms):
                            nc.sync.dma_start(
                                out=avg_d[s][k * 128:(k + 1) * 128,
                                             cx["s0"]:cx["s0"] + NBS],
                                in_=tm if mode == "fp32" else tm.bitcast(wdt))

                for bp in range(nb // 2):
                    cxs = [mk_ctx(0, 2 * bp), mk_ctx(1, 2 * bp + 1)]
                    emit_step0(cxs[0])
                    emit_step0(cxs[1])
                    # interleave the two blocks' steps; halt for step t-1 is
                    # emitted after step t's gate matmuls, off the critical path
                    for step in range(1, T):
                        for cx in cxs:
                            emit_step(cx, step)
                            halt_and_update(cx, step - 1)
                    for cx in cxs:
                        halt_and_update(cx, T - 1)
                        emit_avg(cx)

            # ================= MoE levels =================
            NB2G = min(512, bpc)

            def moe_level(lv, in_drams, in_dt, out_dram, w1_dram, w2_dram,
                          emode, ewdt, fin=None):
                etwo = 2 if emode == "hilo" else 1
                NB2 = NB2G
                nb2 = bpc // NB2
                with tc.tile_pool(name=f"mw{lv}", bufs=1) as wpool, \
                     tc.tile_pool(name=f"min{lv}", bufs=2) as ipool, \
                     tc.tile_pool(name=f"mwk{lv}", bufs=2) as work, \
                     tc.tile_pool(name=f"mc{lv}", bufs=1) as mcp, \
                     tc.tile_pool(name=f"mrt{lv}", bufs=1) as rpool:
                    gateb_lv = mcp.tile([1, E], F32, tag="gb")
                    nc.sync.dma_start(out=gateb_lv[:], in_=gateb_d[lv, :, :])
                    b1_lv = []
                    for e in range(E):
                        ts = []
                        for k in range(KC):
                            t = mcp.tile([128, 1], F32, tag=f"b1_{e}_{k}", name=f"b1_{e}_{k}")
                            nc.sync.dma_start(out=t[:], in_=b1_d[lv, e, k * 128:(k + 1) * 128, :])
                            ts.append(t)
                        b1_lv.append(ts)
                    b2_lv = []
                    for m in range(KC):
                        t = mcp.tile([E, 128], F32, tag=f"b2_{m}", name=f"b2_{m}")
                        nc.sync.dma_start(out=t[:], in_=b2s_d[lv, :, m * 128:(m + 1) * 128])
                        b2_lv.append(t)
                    gw_lv = []
                    for s in range(two):
                        ks = []
                        for k in range(KC):
                            t = mcp.tile([128, E], wdt, tag=f"gw{s}_{k}", name=f"gw{s}_{k}")
                            nc.sync.dma_start(out=t[:], in_=gate_d[lv, s, k * 128:(k + 1) * 128, :])
                            ks.append(t)
                        gw_lv.append(ks)
                    # ---- gates + routing (all blocks) ----
                    w4n_all = []
                    with tc.tile_pool(name=f"rps{lv}", bufs=2, space="PSUM") as rps:
                      for blk in range(nb2):
                        s0 = blk * NB2
                        in6 = []
                        for s in range(two):
                            t = ipool.tile([128, KC, NB2], wdt, tag=f"gin{s}")
                            nc.sync.dma_start(
                                out=t[:],
                                in_=in_drams[s][:, s0:s0 + NB2].rearrange("(k p) s -> p k s", p=128))
                            in6.append(t)
                        ins = [[in6[s][:, k, :] for s in range(two)] for k in range(KC)]
                        w4n = rpool.tile([E, NB2], F32, tag=f"w4n{blk}")
                        for sm in range(NB2 // 128):
                            pg = rps.tile([128, E], F32, tag="pg")
                            pr = []
                            for k in range(KC):
                                lhsT = [ins[k][s][:, sm * 128:(sm + 1) * 128] for s in range(two)]
                                rhs = [gw_lv[s][k][:] for s in range(two)]
                                pr += bld.pairs(lhsT, rhs)
                            bld.mm_group(pg[:], pr, last=False)
                            nc.tensor.matmul(pg[:], ones1[:], gateb_lv[:], start=False, stop=True)
                            lt = work.tile([128, E], F32, tag="lt")
                            nc.vector.tensor_copy(lt[:], pg[:])
                            m1 = work.tile([128, 1], F32, tag="m1")
                            nc.vector.tensor_reduce(m1[:], lt[:], AX.X, ALU.max)
                            eq1 = work.tile([128, E], F32, tag="eq1")
                            nc.vector.tensor_scalar(eq1[:], lt[:], m1[:], None, ALU.is_equal)
                            msk = work.tile([128, E], F32, tag="msk")
                            nc.vector.scalar_tensor_tensor(msk[:], eq1[:], -1e30, lt[:], ALU.mult, ALU.add)
                            m2 = work.tile([128, 1], F32, tag="m2")
                            nc.vector.tensor_reduce(m2[:], msk[:], AX.X, ALU.max)
                            dd = work.tile([128, 1], F32, tag="dd")
                            nc.vector.tensor_sub(dd[:], m1[:], m2[:])
                            sg = work.tile([128, 1], F32, tag="sg")
                            nc.scalar.activation(sg[:], dd[:], AF.Sigmoid)
                            eq2 = work.tile([128, E], F32, tag="eq2")
                            nc.vector.tensor_scalar(eq2[:], msk[:], m2[:], None, ALU.is_equal)
                            oms = work.tile([128, 1], F32, tag="oms")
                            nc.vector.tensor_scalar(oms[:], sg[:], -1.0, 1.0, ALU.mult, ALU.add)
                            wa = work.tile([128, E], F32, tag="wa")
                            nc.vector.tensor_scalar(wa[:], eq1[:], sg[:], None, ALU.mult)
                            wb = work.tile([128, E], F32, tag="wb")
                            nc.vector.tensor_scalar(wb[:], eq2[:], oms[:], None, ALU.mult)
                            wf = work.tile([128, E], F32, tag="wf")
                            nc.vector.tensor_add(wf[:], wa[:], wb[:])
                            ptp = rps.tile([E, 128], F32, tag="ptp")
                            nc.tensor.transpose(ptp[:], wf[:], ident[:])
                            nc.vector.tensor_copy(w4n[:, sm * 128:(sm + 1) * 128], ptp[:])
                        w4n_all.append(w4n)

                    # ---- experts ----
                    with tc.tile_pool(name=f"mps{lv}", bufs=2, space="PSUM") as pspool, \
                         tc.tile_pool(name=f"mpw{lv}", bufs=2, space="PSUM") as psw:
                     for e in range(E):
                        w1_t, w2_t = [], []
                        for s in range(etwo):
                            k1, k2 = [], []
                            for k in range(KC):
                                t1 = wpool.tile([128, H], ewdt, tag=f"w1_{s}_{k}")
                                nc.sync.dma_start(out=t1[:], in_=w1_dram[e, s, k * 128:(k + 1) * 128, :])
                                k1.append(t1)
                                t2 = wpool.tile([128, D], ewdt, tag=f"w2_{s}_{k}")
                                nc.sync.dma_start(out=t2[:], in_=w2_dram[e, s, k * 128:(k + 1) * 128, :])
                                k2.append(t2)
                            w1_t.append(k1)
                            w2_t.append(k2)
                        for blk in range(nb2):
                            s0 = blk * NB2
                            nterm = min(two, etwo) if emode != "fp32" else 1
                            in6 = []
                            for s in range(nterm):
                                t = ipool.tile([128, KC, NB2], wdt, tag=f"gin{s}")
                                nc.sync.dma_start(
                                    out=t[:],
                                    in_=in_drams[s][:, s0:s0 + NB2].rearrange("(k p) s -> p k s", p=128))
                                in6.append(t)
                            ins = [[in6[s][:, k, :] for s in range(nterm)] for k in range(KC)]
                            # w_e broadcast [128, NB2]
                            we_row = work.tile([1, NB2], F32, tag="we_row")
                            nc.sync.dma_start(out=we_row[:], in_=w4n_all[blk][e:e + 1, :])
                            pwb = psw.tile([128, NB2], F32, tag="pwb")
                            nc.tensor.matmul(pwb[:], ones1[:], we_row[:], start=True, stop=True)
                            # W1 stage
                            hm_terms = []
                            for m in range(KC):
                                ph = pspool.tile([128, NB2], F32, tag="ph1")
                                pr = []
                                for k in range(KC):
                                    w_terms = [w1_t[s][k][:, m * 128:(m + 1) * 128] for s in range(etwo)]
                                    pr += bld.pairs(w_terms, ins[k])
                                bld.mm_group(ph[:], pr)
                                hm = work.tile([128, NB2], F32, tag="hm")
                                nc.scalar.activation(hm[:], ph[:], AF.Relu, bias=b1_lv[e][m][:])
                                if emode == "tf32":
                                    # fused weighting + f32r round on the DVE write
                                    hw = work.tile([128, NB2], F32R, tag=f"hwr{m}",
                                                   bufs=1)
                                    nc.vector.tensor_mul(hw[:], hm[:], pwb[:])
                                    hm_terms.append([hw[:]])
                                else:
                                    hw = work.tile([128, NB2], F32, tag="hw")
                                    nc.vector.tensor_mul(hw[:], hm[:], pwb[:])
                                    hm_terms.append(bld.decomp(work, hw[:], f"hwd{m}", emode))
                            # W2 stage
                            for m in range(KC):
                                po = pspool.tile([128, NB2], F32, tag="po2")
                                if e == 0:
                                    nc.tensor.matmul(po[:], b2_lv[m][:], w4n_all[blk][:],
                                                     start=True, stop=False)
                                pr = []
                                for k in range(KC):
                                    w_terms = [w2_t[s][k][:, m * 128:(m + 1) * 128] for s in range(etwo)]
                                    pr += bld.pairs(w_terms, hm_terms[k])
                                bld.mm_group(po[:], pr, first=(e != 0))
                                if fin is not None and e == E - 1:
                                    # read back partials (e0..e2), add, finalize
                                    part = work.tile([128, NB2], F32, tag="part")
                                    nc.sync.dma_start(
                                        out=part[:],
                                        in_=out_dram[m * 128:(m + 1) * 128, s0:s0 + NB2])
                                    fin(work, pspool, m, s0, po[:], part[:])
                                else:
                                    ev = work.tile([128, NB2], F32, tag="oev")
                                    nc.vector.tensor_copy(ev[:], po[:])
                                    if e == 0:
                                        nc.sync.dma_start(out=out_dram[m * 128:(m + 1) * 128, s0:s0 + NB2], in_=ev[:])
                                    else:
                                        nc.gpsimd.dma_start(out=out_dram[m * 128:(m + 1) * 128, s0:s0 + NB2],
                                                            in_=ev[:], accum_op=ALU.add)

            # l0 finalize: round + hi/lo split -> l1 input (no separate phase)
            l1in_d = [nc.dram_tensor(f"l1in_s{i}", [D, bpc], wdt).ap() for i in range(two)]

            def fin_l0(fwork, fpsum, m, s0, po_ap, part_ap):
                tot = fwork.tile([128, NB2G], F32, tag="fintot")
                nc.vector.tensor_add(tot[:], po_ap, part_ap)
                terms = bld.decomp(fwork, tot[:], f"fin{m % 2}", mode)
                for s, tm in enumerate(terms):
                    nc.sync.dma_start(out=l1in_d[s][m * 128:(m + 1) * 128, s0:s0 + NB2G],
                                      in_=tm if mode == "fp32" else tm.bitcast(wdt))

            moe_level(0, avg_d, wdt, l0o_d, w1l0_d, w2l0_d, mode, wdt, fin=fin_l0)

            # l1 finalize: collect the block's 6 chunks, then LayerNorm in
            # sample-major layout (transpose first; per-partition scalars)
            l1tot = {}

            def fin_l1(fwork, fpsum, m, s0, po_ap, part_ap):
                t = fwork.tile([128, NB2G], F32, tag=f"totm{m}", bufs=1)
                nc.vector.tensor_add(t[:], po_ap, part_ap)
                l1tot[m] = t
                if m < KC - 1:
                    return
                outs = [l1tot[k][:] for k in range(KC)]
                for sm in range(NB2G // 128):
                    yb = fwork.tile([128, D], F32, tag="yb")
                    for k in range(KC):
                        ptt = fpsum.tile([128, 128], F32, tag="ptt")
                        nc.tensor.transpose(ptt[:], outs[k][:, sm * 128:(sm + 1) * 128], ident[:])
                        nc.vector.tensor_copy(yb[:, k * 128:(k + 1) * 128], ptt[:])
                    mu = fwork.tile([128, 1], F32, tag="mu")
                    nc.vector.tensor_reduce(mu[:], yb[:], AX.X, ALU.add)
                    nmu = fwork.tile([128, 1], F32, tag="nmu")
                    nc.vector.tensor_scalar(nmu[:], mu[:], -1.0 / D, None, ALU.mult)
                    d_ = fwork.tile([128, D], F32, tag="dln")
                    nc.vector.tensor_scalar(d_[:], yb[:], nmu[:], None, ALU.add)
                    q = fwork.tile([128, D], F32, tag="qln")
                    nc.gpsimd.tensor_mul(q[:], d_[:], d_[:])
                    var = fwork.tile([128, 1], F32, tag="var")
                    nc.vector.tensor_reduce(var[:], q[:], AX.X, ALU.add)
                    sd = fwork.tile([128, 1], F32, tag="sdln")
                    nc.scalar.activation(sd[:], var[:], AF.Sqrt, bias=eps128[:],
                                         scale=1.0 / D)
                    rs = fwork.tile([128, 1], F32, tag="rsln")
                    nc.vector.reciprocal(rs[:], sd[:])
                    zt = fwork.tile([128, D], F32, tag="zt")
                    nc.vector.scalar_tensor_tensor(zt[:], d_[:], rs[:], gammab[:],
                                                   ALU.mult, ALU.mult)
                    obig = fwork.tile([128, D], F32, tag="obig")
                    nc.vector.tensor_add(obig[:], zt[:], betab[:])
                    nc.sync.dma_start(out=out_d[s0 + sm * 128:s0 + (sm + 1) * 128, :],
                                      in_=obig[:])

            moe_level(1, l1in_d, wdt, l1o_d, w1l1_d, w2l1_d, mode_l1, wdt1,
                      fin=fin_l1)

            # ================= LayerNorm + transpose out =================
            with tc.tile_pool(name="ln", bufs=2) as work, \
                 tc.tile_pool(name="lnps", bufs=1, space="PSUM") as pspool, \
                 tc.tile_pool(name="lnpst", bufs=2, space="PSUM") as pst, \
                 tc.tile_pool(name="lnpsb", bufs=1, space="PSUM") as psb, \
                 tc.tile_pool(name="outp", bufs=2) as opool:
                NB2 = min(512, bpc)
                nb2 = bpc // NB2
                gam_t, bet_t = [], []
                for k in range(KC):
                    g = work.tile([128, 1], F32, tag=f"gam{k}", name=f"gam{k}", bufs=1)
                    nc.sync.dma_start(out=g[:], in_=gamma_d[k * 128:(k + 1) * 128, :])
                    gam_t.append(g)
                    bb = work.tile([128, 1], F32, tag=f"bet{k}", name=f"bet{k}", bufs=1)
                    nc.sync.dma_start(out=bb[:], in_=beta_d[k * 128:(k + 1) * 128, :])
                    bet_t.append(bb)
                for blk in range(nb2):
                    s0 = blk * NB2
                    y6 = work.tile([128, KC, NB2], F32, tag="y6")
                    nc.sync.dma_start(
                        out=y6[:],
                        in_=l1o_d[:, s0:s0 + NB2].rearrange("(k p) s -> p k s", p=128))
                    yts = [y6[:, k, :] for k in range(KC)]
                    pmu = pspool.tile([1, NB2], F32, tag="pmu")
                    for k in range(KC):
                        nc.tensor.matmul(pmu[:], oinv[:], yts[k], start=(k == 0), stop=(k == KC - 1))
                    mu = work.tile([1, NB2], F32, tag="mu")
                    nc.vector.tensor_copy(mu[:], pmu[:])
                    pmub = psb.tile([128, NB2], F32, tag="pmub")
                    nc.tensor.matmul(pmub[:], ones1[:], mu[:], start=True, stop=True)
                    dts = []
                    sqs = []
                    for k in range(KC):
                        d_ = work.tile([128, NB2], F32, tag=f"d{k}")
                        nc.vector.tensor_sub(d_[:], yts[k], pmub[:])
                        dts.append(d_)
                        q = work.tile([128, NB2], F32, tag="q")
                        nc.vector.tensor_mul(q[:], d_[:], d_[:])
                        sqs.append(q)
                    pvar = pspool.tile([1, NB2], F32, tag="pvar")
                    for k in range(KC):
                        nc.tensor.matmul(pvar[:], oinv[:], sqs[k][:], start=(k == 0), stop=(k == KC - 1))
                    sd = work.tile([1, NB2], F32, tag="sd")
                    nc.scalar.activation(sd[:], pvar[:], AF.Sqrt, bias=eps_t[:])
                    rs = work.tile([1, NB2], F32, tag="rs")
                    nc.vector.reciprocal(rs[:], sd[:])
                    prsb = psb.tile([128, NB2], F32, tag="prsb")
                    nc.tensor.matmul(prsb[:], ones1[:], rs[:], start=True, stop=True)
                    zts = []
                    for k in range(KC):
                        t = work.tile([128, NB2], F32, tag="sc1")
                        nc.vector.scalar_tensor_tensor(t[:], dts[k][:], gam_t[k][:], prsb[:],
                                                       ALU.mult, ALU.mult)
                        t2 = work.tile([128, NB2], F32, tag=f"z{k}")
                        nc.vector.tensor_scalar(t2[:], t[:], bet_t[k][:], None, ALU.add)
                        zts.append(t2)
                    for sm in range(NB2 // 128):
                        obig = opool.tile([128, D], F32, tag="obig")
                        for k in range(KC):
                            ptt = pst.tile([128, 128], F32, tag="ptt")
                            nc.tensor.transpose(ptt[:], zts[k][:, sm * 128:(sm + 1) * 128], ident[:])
                            nc.vector.tensor_copy(obig[:, k * 128:(k + 1) * 128], ptt[:])
                        nc.sync.dma_start(out=out_d[s0 + sm * 128:s0 + (sm + 1) * 128, :], in_=obig[:])

    nc.compile()
    return nc, names, cfg


_CACHE = {}


def _get_built(cfg_key):
    if cfg_key not in _CACHE:
        cfg = dict(cfg_key)
        _CACHE[cfg_key] = build(cfg)
    return _CACHE[cfg_key]


def prep_shared(inputs, mode="hilo", mode_l1="tf32", mode_lstm=None):
    """Host-side weight prep shared across cores."""
    if mode_lstm is None:
        mode_lstm = mode
    f32 = lambda a: np.ascontiguousarray(a, np.float32)

    def packT(w, m):
        # w: [out, in] -> lhsT [in, out]; stack hi/lo per mode
        wT = f32(w).T
        if m == "fp32":
            return wT[None]
        if m == "tf32":
            return _tf32(np.ascontiguousarray(wT))[None]
        hi, lo = _split(np.ascontiguousarray(wT))
        return np.stack([hi, lo])

    d = {}
    d["wiT"] = packT(inputs["Wi"], mode)
    d["whT"] = packT(inputs["Wh"], mode_lstm)
    d["whaltT"] = packT(inputs["Whalt"], mode_lstm)
    d["bib"] = (f32(inputs["bi"]) + f32(inputs["bh"]))[:, None]
    d["bhalt"] = f32(inputs["bhalt"])[:, None]
    d["gateT"] = np.stack([packT(inputs["gateW"][l], mode) for l in range(LV)])
    d["gateb"] = f32(inputs["gateb"])[:, None, :]
    d["w1l0T"] = np.stack([packT(inputs["W1"][0, e], mode) for e in range(E)])
    d["w2l0T"] = np.stack([packT(inputs["W2"][0, e], mode) for e in range(E)])
    d["w1l1T"] = np.stack([packT(inputs["W1"][1, e], mode_l1) for e in range(E)])
    d["w2l1T"] = np.stack([packT(inputs["W2"][1, e], mode_l1) for e in range(E)])
    d["b1"] = f32(inputs["b1"])[..., None]
    d["b2s"] = f32(inputs["b2"])
    d["gammaT"] = f32(inputs["gamma"])[None, :]
    d["betaT"] = f32(inputs["beta"])[None, :]
    d["ident128"] = np.eye(128, dtype=np.float32)
    return d


LAST_RESULTS = None


def kernel(**inputs):
    global LAST_RESULTS
    mode, mode_l1, mode_lstm = "hilo", "tf32", "tf32"
    cfg_key = (("mode", mode), ("mode_l1", mode_l1),
               ("mode_lstm", mode_lstm), ("nb", NB))
    nc, names, cfg = _get_built(cfg_key)
    shared = prep_shared(inputs, mode, mode_l1, mode_lstm)
    x = np.ascontiguousarray(inputs["x"], np.float32)
    in_maps = []
    for c in range(NCORES):
        m = dict(shared)
        m["x"] = x[c * BPC:(c + 1) * BPC]
        in_maps.append(m)
    res = run_bass_kernel_spmd(nc, in_maps, core_ids=list(range(NCORES)))
    LAST_RESULTS = res
    out = np.concatenate([res.results[c]["out"] for c in range(NCORES)], axis=0)
    return out.astype(np.float32)


if __name__ == "__main__":
    import time
    t0 = time.time()
    nc, names, cfg = build({"mode": "hilo", "mode_l1": "tf32",
                            "mode_lstm": "tf32", "nb": NB})
    print("build+compile ok in", time.time() - t0, "s")



# revision 58
# speedup vs baseline: 1.1181x; 1.1181x over previous
"""MoRALayer Trainium2 kernel.

Data-parallel over 8 NeuronCores (2048 samples each). Per core, feature-major
layout ([feature, sample] tiles): 10-step LSTM ACT loop, 2 sparse-MoE levels
(top-2 of 4 experts), LayerNorm.

Precision scheme (validated against a numpy rounding simulator of the MoE
top-2 routing flip rate): xWi, gate logits and level-0 experts use a hi/lo
tf32 (float32r) split - 3 single-cycle-per-row matmuls recovering ~fp32
accuracy - because their noise feeds the routing decisions (level-0 expert
output drives level-1 gates, whose top2/3rd margins are ~10x tighter than
level-0's). The dominant LSTM recurrence (h@Wh, 10 steps) and the level-1
experts run single-pass tf32: their ~5e-4 noise costs only ~1 routing flip
out of 16k samples (rel err ~7e-3 vs the 2e-2 gate) and cuts tensor time 3x.
Elementwise math and transcendentals are fp32.

Structure: xwi precomputed once (block-major DRAM layout, 4KB lines), held
SBUF-resident per 256-sample block across all 10 LSTM steps; two blocks run
the LSTM in tandem (interleaved step emission, per-parity state) so one
block's cross-engine activation chains hide under the other's matmuls; i/f
and g/o gate pre-activations share packed PSUM banks; SBUF-only elementwise
ops offload to the idle GpSimd engine except on the step-critical last chunk;
per-step halt projections are deferred one step off the inter-step critical
path. Both MoE levels fuse their epilogue into the last expert's evacuation
via partial-sum readback: level 0 rounds/splits the l1 input inline, level 1
runs LayerNorm per block in sample-major layout (transpose first, then
per-partition-scalar normalization), so the LN tail overlaps expert compute.
"""
import os
import sys
import numpy as np

for p in ("/root/.axon_site", "/root/.axon_site/_ro/trn_rl_repo",
          "/root/.axon_site/_ro/pypackages", "/opt/trn_rl_repo"):
    if os.path.isdir(p) and p not in sys.path:
        sys.path.append(p)

import concourse.bass as bass
import concourse.mybir as mybir
import concourse.tile as tile
from concourse import bacc
from concourse.bass_utils import run_bass_kernel_spmd

dt = mybir.dt
AF = mybir.ActivationFunctionType
ALU = mybir.AluOpType
AX = mybir.AxisListType

B, D, H, E, LV, TOPK, T = 16384, 768, 768, 4, 2, 2, 10
LN_EPS = 1e-5
NCORES = 8
BPC = B // NCORES          # samples per core
NBS = 256                  # samples per block
NB = BPC // NBS            # blocks per core
KC = D // 128              # 6 k-chunks over features
GM = 4 * H // 128          # 24 gate m-chunks

F32, F32R = dt.float32, dt.float32r


def _tf32(x):
    u = np.ascontiguousarray(x, np.float32).view(np.uint32)
    u = (u + np.uint32(0x1000)) & np.uint32(0xFFFFE000)
    return u.view(np.float32)


def _split(x):
    hi = _tf32(x)
    lo = _tf32(np.asarray(x, np.float32) - hi)
    return hi, lo


class Builder:
    def __init__(self, nc, tc, ctxpools):
        self.nc = nc
        self.tc = tc

    # ---- helpers -------------------------------------------------------
    def mm_group(self, psum_ap, pairs, first=True, last=True):
        nc = self.nc
        n = len(pairs)
        for i, (l, r) in enumerate(pairs):
            nc.tensor.matmul(psum_ap, l, r,
                             start=(first and i == 0), stop=(last and i == n - 1))

    def decomp(self, pool, val_ap, tag, mode, eng=None):
        """Split an fp32 AP into matmul operand tiles per mode.
        Returns list of rhs aps: hilo -> [hi, lo]; tf32 -> [hi]; fp32 -> [val].
        The hi+lo pair represents val exactly regardless of the engine's
        rounding mode, so eng may be gpsimd to offload the DVE."""
        nc = self.nc
        v = eng if eng is not None else nc.vector
        if mode == "fp32":
            return [val_ap]
        hi = pool.tile(list(val_ap.shape), F32R, tag=tag + "_hi", bufs=1)
        v.tensor_copy(hi[:], val_ap)
        if mode == "tf32":
            return [hi[:]]
        lo = pool.tile(list(val_ap.shape), F32R, tag=tag + "_lo", bufs=1)
        v.tensor_sub(lo[:], val_ap, hi[:].bitcast(F32))
        return [hi[:], lo[:]]

    @staticmethod
    def pairs(w_terms, r_terms):
        """w_terms/r_terms: lists from weight dict / decomp. hilo x hilo ->
        (wh,rh),(wh,rl),(wl,rh); single x single -> 1 pair; fp32 -> 1."""
        if len(w_terms) == 2 and len(r_terms) == 2:
            return [(w_terms[0], r_terms[0]), (w_terms[0], r_terms[1]),
                    (w_terms[1], r_terms[0])]
        if len(w_terms) == 1 and len(r_terms) == 1:
            return [(w_terms[0], r_terms[0])]
        if len(w_terms) == 2 and len(r_terms) == 1:
            return [(w_terms[0], r_terms[0]), (w_terms[1], r_terms[0])]
        if len(w_terms) == 1 and len(r_terms) == 2:
            return [(w_terms[0], r_terms[0]), (w_terms[0], r_terms[1])]
        raise ValueError


def build(cfg):
    mode = cfg.get("mode", "hilo")          # xWi/gates/l0 experts
    mode_l1 = cfg.get("mode_l1", "tf32")    # l1 expert matmuls
    mode_lstm = cfg.get("mode_lstm", mode)  # recurrent h@Wh + halt matmuls
    nb = cfg.get("nb", NB)
    bpc = nb * NBS

    nc = bacc.Bacc("TRN2", target_bir_lowering=False, debug=False)
    names = []

    def inp(name, shape, dtyp=F32):
        names.append(name)
        return nc.dram_tensor(name, list(shape), dtyp, kind="ExternalInput").ap()

    two = 2 if mode == "hilo" else 1
    two1 = 2 if mode_l1 == "hilo" else 1
    two_l = 2 if mode_lstm == "hilo" else 1
    wdt_l = F32R if mode_lstm != "fp32" else F32
    # inputs (weights host-transposed; hi/lo host-split when hilo)
    x_d = inp("x", [bpc, D])
    wi_d = inp("wiT", [two, D, 4 * H], F32R if mode != "fp32" else F32)
    wh_d = inp("whT", [two_l, D, 4 * H], wdt_l)
    whalt_d = inp("whaltT", [two_l, D, 1], wdt_l)
    bib_d = inp("bib", [4 * H, 1])         # bi + bh
    bhalt_d = inp("bhalt", [1, 1])
    gate_d = inp("gateT", [LV, two, D, E], F32R if mode != "fp32" else F32)
    gateb_d = inp("gateb", [LV, 1, E])
    w1l0_d = inp("w1l0T", [E, two, H, H], F32R if mode != "fp32" else F32)
    w2l0_d = inp("w2l0T", [E, two, H, D], F32R if mode != "fp32" else F32)
    w1l1_d = inp("w1l1T", [E, two1, H, H], F32R if mode_l1 != "fp32" else F32)
    w2l1_d = inp("w2l1T", [E, two1, H, D], F32R if mode_l1 != "fp32" else F32)
    b1_d = inp("b1", [LV, E, H, 1])
    b2s_d = inp("b2s", [LV, E, D])         # stacked for k=4 matmul
    gammaT_d = inp("gammaT", [1, D])
    betaT_d = inp("betaT", [1, D])
    ident_d = inp("ident128", [128, 128])
    out_d = nc.dram_tensor("out", [bpc, D], F32, kind="ExternalOutput").ap()

    # DRAM scratch (xwi block-major: per-partition lines of 4*NBS floats)
    xwi_d = nc.dram_tensor("xwi_s", [KC, 128, nb, 4, NBS], F32).ap()
    avg_d = [nc.dram_tensor(f"avg_s{i}", [D, bpc], F32R if mode != "fp32" else F32).ap()
             for i in range(two)]
    l0o_d = nc.dram_tensor("l0o_s", [D, bpc], F32).ap()
    l1o_d = nc.dram_tensor("l1o_s", [D, bpc], F32).ap()

    wdt = F32R if mode != "fp32" else F32
    wdt1 = F32R if mode_l1 != "fp32" else F32

    with tile.TileContext(nc) as tc:
        bld = Builder(nc, tc, None)
        with tc.tile_pool(name="const", bufs=1) as cpool:
            ident = cpool.tile([128, 128], F32, tag="ident")
            nc.sync.dma_start(out=ident[:], in_=ident_d[:])
            ones1 = cpool.tile([1, 128], F32, tag="ones1")
            nc.vector.memset(ones1[:], 1.0)
            cst01 = cpool.tile([1, 128], F32, tag="cst01")
            nc.vector.memset(cst01[:], 1.0 / T)
            oinv = cpool.tile([128, 1], F32, tag="oinv")
            nc.vector.memset(oinv[:], 1.0 / D)
            eps_t = cpool.tile([1, 1], F32, tag="epsln")
            nc.vector.memset(eps_t[:], LN_EPS)
            bhalt_t = cpool.tile([1, 1], F32, tag="bhalt")
            nc.sync.dma_start(out=bhalt_t[:], in_=bhalt_d[:, :])
            whalt_t = []
            for s in range(two_l):
                ks = []
                for k in range(KC):
                    t = cpool.tile([128, 1], wdt_l, tag=f"whalt{s}_{k}")
                    nc.sync.dma_start(out=t[:], in_=whalt_d[s, k * 128:(k + 1) * 128, :])
                    ks.append(t)
                whalt_t.append(ks)
            # LN constants (broadcast along partitions for sample-major LN);
            # the staging tiles + psum close before phase A1 opens
            gammab = cpool.tile([128, D], F32, tag="gammab")
            betab = cpool.tile([128, D], F32, tag="betab")
            eps128 = cpool.tile([128, 1], F32, tag="eps128")
            nc.vector.memset(eps128[:], LN_EPS)
            with tc.tile_pool(name="lnst", bufs=1) as stp, \
                 tc.tile_pool(name="lnbc", bufs=2, space="PSUM") as bcp:
                gT = stp.tile([1, D], F32, tag="gT")
                nc.sync.dma_start(out=gT[:], in_=gammaT_d[:, :])
                bT = stp.tile([1, D], F32, tag="bT")
                nc.sync.dma_start(out=bT[:], in_=betaT_d[:, :])
                for j in range(0, D, 512):
                    w = min(512, D - j)
                    pb = bcp.tile([128, 512], F32, tag="bc")
                    nc.tensor.matmul(pb[:, :w], ones1[:], gT[:, j:j + w], start=True, stop=True)
                    nc.vector.tensor_copy(gammab[:, j:j + w], pb[:, :w])
                    pb2 = bcp.tile([128, 512], F32, tag="bc")
                    nc.tensor.matmul(pb2[:, :w], ones1[:], bT[:, j:j + w], start=True, stop=True)
                    nc.vector.tensor_copy(betab[:, j:j + w], pb2[:, :w])

            # ================= phase A1: xWi =================
            with tc.tile_pool(name="wiA", bufs=1) as wpool, \
                 tc.tile_pool(name="a1work", bufs=2) as work, \
                 tc.tile_pool(name="a1ps", bufs=3, space="PSUM") as pspool:
                wi_t = []
                for s in range(two):
                    ks = []
                    for k in range(KC):
                        t = wpool.tile([128, 4 * H], wdt, tag=f"wi{s}_{k}")
                        nc.sync.dma_start(out=t[:], in_=wi_d[s, k * 128:(k + 1) * 128, :])
                        ks.append(t)
                    wi_t.append(ks)
                bib_t = []
                for m in range(GM):
                    t = wpool.tile([128, 1], F32, tag=f"bib{m}", name=f"bib{m}")
                    nc.sync.dma_start(out=t[:], in_=bib_d[m * 128:(m + 1) * 128, :])
                    bib_t.append(t)
                for blk in range(nb):
                    s0 = blk * NBS
                    xsm = []
                    for sm in range(NBS // 128):
                        t = work.tile([128, D], F32, tag=f"xsm{sm}")
                        nc.sync.dma_start(out=t[:], in_=x_d[s0 + sm * 128:s0 + (sm + 1) * 128, :])
                        xsm.append(t)
                    xts = []
                    for k in range(KC):
                        xt = work.tile([128, NBS], F32, tag=f"xT{k}")
                        for sm in range(NBS // 128):
                            ptx = pspool.tile([128, 128], F32, tag="ptx")
                            nc.tensor.transpose(ptx[:], xsm[sm][:, k * 128:(k + 1) * 128], ident[:])
                            nc.vector.tensor_copy(xt[:, sm * 128:(sm + 1) * 128], ptx[:])
                        xts.append(bld.decomp(work, xt[:], f"xT{k}", mode))
                    for m in range(GM):
                        ps = pspool.tile([128, NBS], F32, tag="a1")
                        pr = []
                        for k in range(KC):
                            w_terms = [wi_t[s][k][:, m * 128:(m + 1) * 128] for s in range(two)]
                            pr += bld.pairs(w_terms, xts[k])
                        bld.mm_group(ps[:], pr)
                        ev = work.tile([128, NBS], F32, tag="a1ev")
                        nc.vector.tensor_scalar(ev[:], ps[:], bib_t[m][:], None, ALU.add)
                        nc.sync.dma_start(out=xwi_d[m % KC, :, blk, m // KC, :], in_=ev[:])

            # ================= phase A2: LSTM =================
            with tc.tile_pool(name="whA", bufs=1) as wpool, \
                 tc.tile_pool(name="xwip", bufs=2) as xpool, \
                 tc.tile_pool(name="state", bufs=1) as spool, \
                 tc.tile_pool(name="a2work", bufs=2) as work, \
                 tc.tile_pool(name="a2act", bufs=1) as apool, \
                 tc.tile_pool(name="a2ps", bufs=3, space="PSUM") as pspool, \
                 tc.tile_pool(name="a2psr", bufs=1, space="PSUM") as psrem, \
                 tc.tile_pool(name="a2psh", bufs=1, space="PSUM") as pshalt:
                wh_t = []
                for s in range(two_l):
                    ks = []
                    for k in range(KC):
                        t = wpool.tile([128, 4 * H], wdt_l, tag=f"wh{s}_{k}")
                        nc.sync.dma_start(out=t[:], in_=wh_d[s, k * 128:(k + 1) * 128, :])
                        ks.append(t)
                    wh_t.append(ks)

                hdt = F32 if mode_lstm == "fp32" else F32R

                def mk_ctx(par, blk):
                    # per-parity state: two blocks fly in tandem so one
                    # block's cross-engine chains hide under the other's mms
                    xwi_sb = xpool.tile([128, KC, 4, NBS], F32,
                                        tag=f"xwisb{par}", bufs=1)
                    nc.sync.dma_start(
                        out=xwi_sb[:],
                        in_=xwi_d[:, :, blk, :, :].rearrange("k p g s -> p k g s"))
                    tl = lambda shape, dty, tg: spool.tile(
                        shape, dty, tag=tg, name=tg)
                    return {
                        "s0": blk * NBS, "xwi": xwi_sb,
                        "c": [tl([128, NBS], F32, f"c{k}_{par}") for k in range(KC)],
                        "hs": [tl([128, NBS], F32, f"hs{k}_{par}") for k in range(KC)],
                        "hh": [[tl([128, NBS], hdt, f"hh{b}_{k}_{par}")
                                for k in range(KC)] for b in range(2)],
                        "hl": [[tl([128, NBS], F32R, f"hl{b}_{k}_{par}")
                                for k in range(KC)] for b in range(2)]
                              if mode_lstm == "hilo" else [[None] * KC] * 2,
                        "hp": tl([1, NBS], F32, f"hp{par}"),
                        "rem": tl([1, NBS], F32, f"rem{par}"),
                    }

                def h_terms(cx, k, b):
                    if mode_lstm == "hilo":
                        return [cx["hh"][b][k][:], cx["hl"][b][k][:]]
                    return [cx["hh"][b][k][:]]

                def store_h(cx, k, b, t_ap):
                    # the last chunk gates the next step's matmuls - keep its
                    # chain on the faster DVE
                    eng = nc.vector if k == KC - 1 else nc.gpsimd
                    eng.tensor_copy(cx["hh"][b][k][:], t_ap)
                    if mode_lstm == "hilo":
                        nc.vector.tensor_sub(cx["hl"][b][k][:], t_ap,
                                             cx["hh"][b][k][:].bitcast(F32))

                def halt_and_update(cx, step):
                    psh = pshalt.tile([1, NBS], F32, tag="halt")
                    pr = []
                    for k in range(KC):
                        w_terms = [whalt_t[s][k][:] for s in range(two_l)]
                        pr += bld.pairs(w_terms, h_terms(cx, k, step % 2))
                    bld.mm_group(psh[:], pr)
                    y = work.tile([1, NBS], F32, tag="y", bufs=1)
                    nc.scalar.activation(y[:], psh[:], AF.Sigmoid, bias=bhalt_t[:])
                    hp, rem = cx["hp"], cx["rem"]
                    if step == 0:
                        nc.vector.tensor_copy(hp[:], y[:])
                        nc.vector.tensor_scalar(rem[:], y[:], -1.0, 1.0, ALU.mult, ALU.add)
                    else:
                        t2 = work.tile([1, NBS], F32, tag="t2", bufs=1)
                        nc.vector.tensor_scalar(t2[:], hp[:], -1.0, 1.0, ALU.mult, ALU.add)
                        t3 = work.tile([1, NBS], F32, tag="t3", bufs=1)
                        nc.vector.tensor_mul(t3[:], t2[:], y[:])
                        nc.vector.tensor_add(hp[:], hp[:], t3[:])
                        t4 = work.tile([1, NBS], F32, tag="t4", bufs=1)
                        nc.vector.tensor_scalar(t4[:], hp[:], -1.0, 1.0, ALU.mult, ALU.add)
                        nc.vector.tensor_add(rem[:], rem[:], t4[:])

                def emit_step0(cx):
                    # h=c=0; gates = xwi directly
                    for k in range(KC):
                        sif = apool.tile([128, 2, NBS], F32, tag="sif", bufs=2)
                        nc.scalar.activation(sif[:, 0, :], cx["xwi"][:, k, 0, :], AF.Sigmoid)
                        tg = apool.tile([128, NBS], F32, tag="tg", bufs=2)
                        nc.scalar.activation(tg[:], cx["xwi"][:, k, 2, :], AF.Tanh)
                        so = apool.tile([128, NBS], F32, tag="so", bufs=2)
                        nc.scalar.activation(so[:], cx["xwi"][:, k, 3, :], AF.Sigmoid)
                        nc.vector.tensor_mul(cx["c"][k][:], sif[:, 0, :], tg[:])
                        th = work.tile([128, NBS], F32, tag="th")
                        nc.scalar.activation(th[:], cx["c"][k][:], AF.Tanh)
                        nc.vector.tensor_mul(th[:], so[:], th[:])
                        store_h(cx, k, 0, th[:])
                        nc.vector.tensor_copy(cx["hs"][k][:], th[:])

                def emit_step(cx, step):
                    for k in range(KC):
                        # gates i,f packed in one PSUM bank; g,o in another
                        ps_if = pspool.tile([128, 2, NBS], F32, tag="psif")
                        ps_go = pspool.tile([128, 2, NBS], F32, tag="psgo")
                        for j, gi in enumerate((0, 1)):
                            m = 6 * gi + k
                            pr = []
                            for kk in range(KC):
                                w_terms = [wh_t[s][kk][:, m * 128:(m + 1) * 128] for s in range(two_l)]
                                pr += bld.pairs(w_terms, h_terms(cx, kk, (step - 1) % 2))
                            bld.mm_group(ps_if[:, j, :], pr)
                        for j, gi in enumerate((2, 3)):
                            m = 6 * gi + k
                            pr = []
                            for kk in range(KC):
                                w_terms = [wh_t[s][kk][:, m * 128:(m + 1) * 128] for s in range(two_l)]
                                pr += bld.pairs(w_terms, h_terms(cx, kk, (step - 1) % 2))
                            bld.mm_group(ps_go[:, j, :], pr)
                        gif = work.tile([128, 2, NBS], F32, tag="gif")
                        nc.vector.tensor_add(gif[:], ps_if[:], cx["xwi"][:, k, 0:2, :])
                        sif = apool.tile([128, 2, NBS], F32, tag="sif", bufs=2)
                        nc.scalar.activation(sif[:], gif[:], AF.Sigmoid)
                        ggo = work.tile([128, 2, NBS], F32, tag="ggo")
                        nc.vector.tensor_add(ggo[:], ps_go[:], cx["xwi"][:, k, 2:4, :])
                        tg = apool.tile([128, NBS], F32, tag="tg", bufs=2)
                        nc.scalar.activation(tg[:], ggo[:, 0, :], AF.Tanh)
                        so = apool.tile([128, NBS], F32, tag="so", bufs=2)
                        nc.scalar.activation(so[:], ggo[:, 1, :], AF.Sigmoid)
                        t1 = work.tile([128, NBS], F32, tag="t1g", bufs=2)
                        teng = nc.vector if k == KC - 1 else nc.gpsimd
                        teng.tensor_mul(t1[:], sif[:, 0, :], tg[:])
                        nc.vector.tensor_mul(cx["c"][k][:], cx["c"][k][:], sif[:, 1, :])
                        nc.vector.tensor_add(cx["c"][k][:], cx["c"][k][:], t1[:])
                        th = work.tile([128, NBS], F32, tag="th")
                        nc.scalar.activation(th[:], cx["c"][k][:], AF.Tanh)
                        nc.vector.tensor_mul(th[:], so[:], th[:])
                        store_h(cx, k, step % 2, th[:])
                        nc.gpsimd.tensor_add(cx["hs"][k][:], cx["hs"][k][:], th[:])

                def emit_avg(cx):
                    # avg = rem * hsum / T -> DRAM (hi/lo)
                    psb = psrem.tile([128, NBS], F32, tag="remb")
                    nc.tensor.matmul(psb[:], cst01[:], cx["rem"][:], start=True, stop=True)
                    for k in range(KC):
                        av = work.tile([128, NBS], F32, tag="av", bufs=2)
                        nc.vector.tensor_mul(av[:], cx["hs"][k][:], psb[:])
                        terms = bld.decomp(work, av[:], "avd", mode)
                        for s, tm in enumerate(terms):
                            nc.sync.dma_start(
                                out=avg_d[s][k * 128:(k + 1) * 128,
                                             cx["s0"]:cx["s0"] + NBS],
                                in_=tm if mode == "fp32" else tm.bitcast(wdt))

                for bp in range(nb // 2):
                    cxs = [mk_ctx(0, 2 * bp), mk_ctx(1, 2 * bp + 1)]
                    emit_step0(cxs[0])
                    emit_step0(cxs[1])
                    # interleave the two blocks' steps; halt for step t-1 is
                    # emitted after step t's gate matmuls, off the critical path
                    for step in range(1, T):
                        for cx in cxs:
                            emit_step(cx, step)
                            halt_and_update(cx, step - 1)
                    for cx in cxs:
                        halt_and_update(cx, T - 1)
                        emit_avg(cx)

            # ================= MoE levels =================
            NB2G = min(512, bpc)

            def moe_level(lv, in_drams, in_dt, out_dram, w1_dram, w2_dram,
                          emode, ewdt, fin=None):
                etwo = 2 if emode == "hilo" else 1
                NB2 = NB2G
                nb2 = bpc // NB2
                with tc.tile_pool(name=f"mw{lv}", bufs=1) as wpool, \
                     tc.tile_pool(name=f"min{lv}", bufs=2) as ipool, \
                     tc.tile_pool(name=f"mwk{lv}", bufs=2) as work, \
                     tc.tile_pool(name=f"mc{lv}", bufs=1) as mcp, \
                     tc.tile_pool(name=f"mrt{lv}", bufs=1) as rpool:
                    gateb_lv = mcp.tile([1, E], F32, tag="gb")
                    nc.sync.dma_start(out=gateb_lv[:], in_=gateb_d[lv, :, :])
                    b1_lv = []
                    for e in range(E):
                        ts = []
                        for k in range(KC):
                            t = mcp.tile([128, 1], F32, tag=f"b1_{e}_{k}", name=f"b1_{e}_{k}")
                            nc.sync.dma_start(out=t[:], in_=b1_d[lv, e, k * 128:(k + 1) * 128, :])
                            ts.append(t)
                        b1_lv.append(ts)
                    b2_lv = []
                    for m in range(KC):
                        t = mcp.tile([E, 128], F32, tag=f"b2_{m}", name=f"b2_{m}")
                        nc.sync.dma_start(out=t[:], in_=b2s_d[lv, :, m * 128:(m + 1) * 128])
                        b2_lv.append(t)
                    gw_lv = []
                    for s in range(two):
                        ks = []
                        for k in range(KC):
                            t = mcp.tile([128, E], wdt, tag=f"gw{s}_{k}", name=f"gw{s}_{k}")
                            nc.sync.dma_start(out=t[:], in_=gate_d[lv, s, k * 128:(k + 1) * 128, :])
                            ks.append(t)
                        gw_lv.append(ks)
                    # ---- gates + routing (all blocks) ----
                    w4n_all = []
                    with tc.tile_pool(name=f"rps{lv}", bufs=2, space="PSUM") as rps:
                      for blk in range(nb2):
                        s0 = blk * NB2
                        in6 = []
                        for s in range(two):
                            t = ipool.tile([128, KC, NB2], wdt, tag=f"gin{s}")
                            nc.sync.dma_start(
                                out=t[:],
                                in_=in_drams[s][:, s0:s0 + NB2].rearrange("(k p) s -> p k s", p=128))
                            in6.append(t)
                        ins = [[in6[s][:, k, :] for s in range(two)] for k in range(KC)]
                        w4n = rpool.tile([E, NB2], F32, tag=f"w4n{blk}")
                        for sm in range(NB2 // 128):
                            pg = rps.tile([128, E], F32, tag="pg")
                            pr = []
                            for k in range(KC):
                                lhsT = [ins[k][s][:, sm * 128:(sm + 1) * 128] for s in range(two)]
                                rhs = [gw_lv[s][k][:] for s in range(two)]
                                pr += bld.pairs(lhsT, rhs)
                            bld.mm_group(pg[:], pr, last=False)
                            nc.tensor.matmul(pg[:], ones1[:], gateb_lv[:], start=False, stop=True)
                            lt = work.tile([128, E], F32, tag="lt")
                            nc.vector.tensor_copy(lt[:], pg[:])
                            m1 = work.tile([128, 1], F32, tag="m1")
                            nc.vector.tensor_reduce(m1[:], lt[:], AX.X, ALU.max)
                            eq1 = work.tile([128, E], F32, tag="eq1")
                            nc.vector.tensor_scalar(eq1[:], lt[:], m1[:], None, ALU.is_equal)
                            msk = work.tile([128, E], F32, tag="msk")
                            nc.vector.scalar_tensor_tensor(msk[:], eq1[:], -1e30, lt[:], ALU.mult, ALU.add)
                            m2 = work.tile([128, 1], F32, tag="m2")
                            nc.vector.tensor_reduce(m2[:], msk[:], AX.X, ALU.max)
                            dd = work.tile([128, 1], F32, tag="dd")
                            nc.vector.tensor_sub(dd[:], m1[:], m2[:])
                            sg = work.tile([128, 1], F32, tag="sg")
                            nc.scalar.activation(sg[:], dd[:], AF.Sigmoid)
                            eq2 = work.tile([128, E], F32, tag="eq2")
                            nc.vector.tensor_scalar(eq2[:], msk[:], m2[:], None, ALU.is_equal)
                            oms = work.tile([128, 1], F32, tag="oms")
                            nc.vector.tensor_scalar(oms[:], sg[:], -1.0, 1.0, ALU.mult, ALU.add)
                            wa = work.tile([128, E], F32, tag="wa")
                            nc.vector.tensor_scalar(wa[:], eq1[:], sg[:], None, ALU.mult)
                            wb = work.tile([128, E], F32, tag="wb")
                            nc.vector.tensor_scalar(wb[:], eq2[:], oms[:], None, ALU.mult)
                            wf = work.tile([128, E], F32, tag="wf")
                            nc.vector.tensor_add(wf[:], wa[:], wb[:])
                            ptp = rps.tile([E, 128], F32, tag="ptp")
                            nc.tensor.transpose(ptp[:], wf[:], ident[:])
                            nc.vector.tensor_copy(w4n[:, sm * 128:(sm + 1) * 128], ptp[:])
                        w4n_all.append(w4n)

                    # ---- experts ----
                    with tc.tile_pool(name=f"mps{lv}", bufs=2, space="PSUM") as pspool, \
                         tc.tile_pool(name=f"mpw{lv}", bufs=2, space="PSUM") as psw:
                     for e in range(E):
                        w1_t, w2_t = [], []
                        for s in range(etwo):
                            k1, k2 = [], []
                            for k in range(KC):
                                t1 = wpool.tile([128, H], ewdt, tag=f"w1_{s}_{k}")
                                nc.sync.dma_start(out=t1[:], in_=w1_dram[e, s, k * 128:(k + 1) * 128, :])
                                k1.append(t1)
                                t2 = wpool.tile([128, D], ewdt, tag=f"w2_{s}_{k}")
                                nc.sync.dma_start(out=t2[:], in_=w2_dram[e, s, k * 128:(k + 1) * 128, :])
                                k2.append(t2)
                            w1_t.append(k1)
                            w2_t.append(k2)
                        for blk in range(nb2):
                            s0 = blk * NB2
                            nterm = min(two, etwo) if emode != "fp32" else 1
                            in6 = []
                            for s in range(nterm):
                                t = ipool.tile([128, KC, NB2], wdt, tag=f"gin{s}")
                                nc.sync.dma_start(
                                    out=t[:],
                                    in_=in_drams[s][:, s0:s0 + NB2].rearrange("(k p) s -> p k s", p=128))
                                in6.append(t)
                            ins = [[in6[s][:, k, :] for s in range(nterm)] for k in range(KC)]
                            # w_e broadcast [128, NB2]
                            we_row = work.tile([1, NB2], F32, tag="we_row")
                            nc.sync.dma_start(out=we_row[:], in_=w4n_all[blk][e:e + 1, :])
                            pwb = psw.tile([128, NB2], F32, tag="pwb")
                            nc.tensor.matmul(pwb[:], ones1[:], we_row[:], start=True, stop=True)
                            # W1 stage
                            hm_terms = []
                            for m in range(KC):
                                ph = pspool.tile([128, NB2], F32, tag="ph1")
                                pr = []
                                for k in range(KC):
                                    w_terms = [w1_t[s][k][:, m * 128:(m + 1) * 128] for s in range(etwo)]
                                    pr += bld.pairs(w_terms, ins[k])
                                bld.mm_group(ph[:], pr)
                                hm = work.tile([128, NB2], F32, tag="hm")
                                nc.scalar.activation(hm[:], ph[:], AF.Relu, bias=b1_lv[e][m][:])
                                if emode == "tf32":
                                    # fused weighting + f32r round on the DVE write
                                    hw = work.tile([128, NB2], F32R, tag=f"hwr{m}",
                                                   bufs=1)
                                    nc.vector.tensor_mul(hw[:], hm[:], pwb[:])
                                    hm_terms.append([hw[:]])
                                else:
                                    hw = work.tile([128, NB2], F32, tag="hw")
                                    nc.vector.tensor_mul(hw[:], hm[:], pwb[:])
                                    hm_terms.append(bld.decomp(work, hw[:], f"hwd{m}", emode))
                            # W2 stage
                            for m in range(KC):
                                po = pspool.tile([128, NB2], F32, tag="po2")
                                if e == 0:
                                    nc.tensor.matmul(po[:], b2_lv[m][:], w4n_all[blk][:],
                                                     start=True, stop=False)
                                pr = []
                                for k in range(KC):
                                    w_terms = [w2_t[s][k][:, m * 128:(m + 1) * 128] for s in range(etwo)]
                                    pr += bld.pairs(w_terms, hm_terms[k])
                                bld.mm_group(po[:], pr, first=(e != 0))
                                if fin is not None and e == E - 1:
                                    # read back partials (e0..e2), add, finalize
                                    part = work.tile([128, NB2], F32, tag="part")
                                    nc.sync.dma_start(
                                        out=part[:],
                                        in_=out_dram[m * 128:(m + 1) * 128, s0:s0 + NB2])
                                    fin(work, pspool, m, s0, po[:], part[:])
                                else:
                                    ev = work.tile([128, NB2], F32, tag="oev")
                                    nc.vector.tensor_copy(ev[:], po[:])
                                    if e == 0:
                                        nc.sync.dma_start(out=out_dram[m * 128:(m + 1) * 128, s0:s0 + NB2], in_=ev[:])
                                    else:
                                        nc.gpsimd.dma_start(out=out_dram[m * 128:(m + 1) * 128, s0:s0 + NB2],
                                                            in_=ev[:], accum_op=ALU.add)

            # l0 finalize: round + hi/lo split -> l1 input (no separate phase)
            l1in_d = [nc.dram_tensor(f"l1in_s{i}", [D, bpc], wdt).ap() for i in range(two)]

            def fin_l0(fwork, fpsum, m, s0, po_ap, part_ap):
                tot = fwork.tile([128, NB2G], F32, tag="fintot")
                nc.vector.tensor_add(tot[:], po_ap, part_ap)
                terms = bld.decomp(fwork, tot[:], f"fin{m % 2}", mode)
                for s, tm in enumerate(terms):
                    nc.sync.dma_start(out=l1in_d[s][m * 128:(m + 1) * 128, s0:s0 + NB2G],
                                      in_=tm if mode == "fp32" else tm.bitcast(wdt))

            moe_level(0, avg_d, wdt, l0o_d, w1l0_d, w2l0_d, mode, wdt, fin=fin_l0)

            # l1 finalize: collect the block's 6 chunks, then LayerNorm in
            # sample-major layout (transpose first; per-partition scalars)
            l1tot = {}

            def fin_l1(fwork, fpsum, m, s0, po_ap, part_ap):
                t = fwork.tile([128, NB2G], F32, tag=f"totm{m}", bufs=1)
                nc.vector.tensor_add(t[:], po_ap, part_ap)
                l1tot[m] = t
                if m < KC - 1:
                    return
                outs = [l1tot[k][:] for k in range(KC)]
                for sm in range(NB2G // 128):
                    yb = fwork.tile([128, D], F32, tag="yb")
                    for k in range(KC):
                        ptt = fpsum.tile([128, 128], F32, tag="ptt")
                        nc.tensor.transpose(ptt[:], outs[k][:, sm * 128:(sm + 1) * 128], ident[:])
                        nc.vector.tensor_copy(yb[:, k * 128:(k + 1) * 128], ptt[:])
                    mu = fwork.tile([128, 1], F32, tag="mu")
                    nc.vector.tensor_reduce(mu[:], yb[:], AX.X, ALU.add)
                    nmu = fwork.tile([128, 1], F32, tag="nmu")
                    nc.vector.tensor_scalar(nmu[:], mu[:], -1.0 / D, None, ALU.mult)
                    d_ = fwork.tile([128, D], F32, tag="dln")
                    nc.vector.tensor_scalar(d_[:], yb[:], nmu[:], None, ALU.add)
                    q = fwork.tile([128, D], F32, tag="qln")
                    nc.gpsimd.tensor_mul(q[:], d_[:], d_[:])
                    var = fwork.tile([128, 1], F32, tag="var")
                    nc.vector.tensor_reduce(var[:], q[:], AX.X, ALU.add)
                    sd = fwork.tile([128, 1], F32, tag="sdln")
                    nc.scalar.activation(sd[:], var[:], AF.Sqrt, bias=eps128[:],
                                         scale=1.0 / D)
                    rs = fwork.tile([128, 1], F32, tag="rsln")
                    nc.vector.reciprocal(rs[:], sd[:])
                    zt = fwork.tile([128, D], F32, tag="zt")
                    nc.vector.scalar_tensor_tensor(zt[:], d_[:], rs[:], gammab[:],
                                                   ALU.mult, ALU.mult)
                    obig = fwork.tile([128, D], F32, tag="obig")
                    nc.vector.tensor_add(obig[:], zt[:], betab[:])
                    nc.sync.dma_start(out=out_d[s0 + sm * 128:s0 + (sm + 1) * 128, :],
                                      in_=obig[:])

            moe_level(1, l1in_d, wdt, l1o_d, w1l1_d, w2l1_d, mode_l1, wdt1,
                      fin=fin_l1)

            # ================= LayerNorm + transpose out =================
            with tc.tile_pool(name="ln", bufs=2) as work, \
                 tc.tile_pool(name="lnps", bufs=1, space="PSUM") as pspool, \
                 tc.tile_pool(name="lnpst", bufs=2, space="PSUM") as pst, \
                 tc.tile_pool(name="lnpsb", bufs=1, space="PSUM") as psb, \
                 tc.tile_pool(name="outp", bufs=2) as opool:
                NB2 = min(512, bpc)
                nb2 = bpc // NB2
                gam_t, bet_t = [], []
                for k in range(KC):
                    g = work.tile([128, 1], F32, tag=f"gam{k}", name=f"gam{k}", bufs=1)
                    nc.sync.dma_start(out=g[:], in_=gamma_d[k * 128:(k + 1) * 128, :])
                    gam_t.append(g)
                    bb = work.tile([128, 1], F32, tag=f"bet{k}", name=f"bet{k}", bufs=1)
                    nc.sync.dma_start(out=bb[:], in_=beta_d[k * 128:(k + 1) * 128, :])
                    bet_t.append(bb)
                for blk in range(nb2):
                    s0 = blk * NB2
                    y6 = work.tile([128, KC, NB2], F32, tag="y6")
                    nc.sync.dma_start(
                        out=y6[:],
                        in_=l1o_d[:, s0:s0 + NB2].rearrange("(k p) s -> p k s", p=128))
                    yts = [y6[:, k, :] for k in range(KC)]
                    pmu = pspool.tile([1, NB2], F32, tag="pmu")
                    for k in range(KC):
                        nc.tensor.matmul(pmu[:], oinv[:], yts[k], start=(k == 0), stop=(k == KC - 1))
                    mu = work.tile([1, NB2], F32, tag="mu")
                    nc.vector.tensor_copy(mu[:], pmu[:])
                    pmub = psb.tile([128, NB2], F32, tag="pmub")
                    nc.tensor.matmul(pmub[:], ones1[:], mu[:], start=True, stop=True)
                    dts = []
                    sqs = []
                    for k in range(KC):
                        d_ = work.tile([128, NB2], F32, tag=f"d{k}")
                        nc.vector.tensor_sub(d_[:], yts[k], pmub[:])
                        dts.append(d_)
                        q = work.tile([128, NB2], F32, tag="q")
                        nc.vector.tensor_mul(q[:], d_[:], d_[:])
                        sqs.append(q)
                    pvar = pspool.tile([1, NB2], F32, tag="pvar")
                    for k in range(KC):
                        nc.tensor.matmul(pvar[:], oinv[:], sqs[k][:], start=(k == 0), stop=(k == KC - 1))
                    sd = work.tile([1, NB2], F32, tag="sd")
                    nc.scalar.activation(sd[:], pvar[:], AF.Sqrt, bias=eps_t[:])
                    rs = work.tile([1, NB2], F32, tag="rs")
                    nc.vector.reciprocal(rs[:], sd[:])
                    prsb = psb.tile([128, NB2], F32, tag="prsb")
                    nc.tensor.matmul(prsb[:], ones1[:], rs[:], start=True, stop=True)
                    zts = []
                    for k in range(KC):
                        t = work.tile([128, NB2], F32, tag="sc1")
                        nc.vector.scalar_tensor_tensor(t[:], dts[k][:], gam_t[k][:], prsb[:],
                                                       ALU.mult, ALU.mult)
                        t2 = work.tile([128, NB2], F32, tag=f"z{k}")
                        nc.vector.tensor_scalar(t2[:], t[:], bet_t[k][:], None, ALU.add)
                        zts.append(t2)
                    for sm in range(NB2 // 128):
                        obig = opool.tile([128, D], F32, tag="obig")
                        for k in range(KC):
                            ptt = pst.tile([128, 128], F32, tag="ptt")
                            nc.tensor.transpose(ptt[:], zts[k][:, sm * 128:(sm + 1) * 128], ident[:])
                            nc.vector.tensor_copy(obig[:, k * 128:(k + 1) * 128], ptt[:])
                        nc.sync.dma_start(out=out_d[s0 + sm * 128:s0 + (sm + 1) * 128, :], in_=obig[:])

    nc.compile()
    return nc, names, cfg


_CACHE = {}


def _get_built(cfg_key):
    if cfg_key not in _CACHE:
        cfg = dict(cfg_key)
        _CACHE[cfg_key] = build(cfg)
    return _CACHE[cfg_key]


def prep_shared(inputs, mode="hilo", mode_l1="tf32", mode_lstm=None):
    """Host-side weight prep shared across cores."""
    if mode_lstm is None:
        mode_lstm = mode
    f32 = lambda a: np.ascontiguousarray(a, np.float32)

    def packT(w, m):
        # w: [out, in] -> lhsT [in, out]; stack hi/lo per mode
        wT = f32(w).T
        if m == "fp32":
            return wT[None]
        if m == "tf32":
            return _tf32(np.ascontiguousarray(wT))[None]
        hi, lo = _split(np.ascontiguousarray(wT))
        return np.stack([hi, lo])

    d = {}
    d["wiT"] = packT(inputs["Wi"], mode)
    d["whT"] = packT(inputs["Wh"], mode_lstm)
    d["whaltT"] = packT(inputs["Whalt"], mode_lstm)
    d["bib"] = (f32(inputs["bi"]) + f32(inputs["bh"]))[:, None]
    d["bhalt"] = f32(inputs["bhalt"])[:, None]
    d["gateT"] = np.stack([packT(inputs["gateW"][l], mode) for l in range(LV)])
    d["gateb"] = f32(inputs["gateb"])[:, None, :]
    d["w1l0T"] = np.stack([packT(inputs["W1"][0, e], mode) for e in range(E)])
    d["w2l0T"] = np.stack([packT(inputs["W2"][0, e], mode) for e in range(E)])
    d["w1l1T"] = np.stack([packT(inputs["W1"][1, e], mode_l1) for e in range(E)])
    d["w2l1T"] = np.stack([packT(inputs["W2"][1, e], mode_l1) for e in range(E)])
    d["b1"] = f32(inputs["b1"])[..., None]
    d["b2s"] = f32(inputs["b2"])
    d["gammaT"] = f32(inputs["gamma"])[None, :]
    d["betaT"] = f32(inputs["beta"])[None, :]
    d["ident128"] = np.eye(128, dtype=np.float32)
    return d


LAST_RESULTS = None


def kernel(**inputs):
    global LAST_RESULTS
    mode, mode_l1, mode_lstm = "hilo", "tf32", "tf32"
    cfg_key = (("mode", mode), ("mode_l1", mode_l1),
               ("mode_lstm", mode_lstm), ("nb", NB))
    nc, names, cfg = _get_built(cfg_key)
    shared = prep_shared(inputs, mode, mode_l1, mode_lstm)
    x = np.ascontiguousarray(inputs["x"], np.float32)
    in_maps = []
    for c in range(NCORES):
        m = dict(shared)
        m["x"] = x[c * BPC:(c + 1) * BPC]
        in_maps.append(m)
    res = run_bass_kernel_spmd(nc, in_maps, core_ids=list(range(NCORES)))
    LAST_RESULTS = res
    out = np.concatenate([res.results[c]["out"] for c in range(NCORES)], axis=0)
    return out.astype(np.float32)


if __name__ == "__main__":
    import time
    t0 = time.time()
    nc, names, cfg = build({"mode": "hilo", "mode_l1": "tf32",
                            "mode_lstm": "tf32", "nb": NB})
    print("build+compile ok in", time.time() - t0, "s")

